# revision 15
# baseline (speedup 1.0000x reference)
"""CRF log-likelihood loss kernel for Trainium2 (8 NeuronCores, Bass/Tile).

Strategy (data-parallel over batch, per sharding hint):
  - B=256 batch rows sharded 32 per core; W/b/CRF tables replicated.
  - Host pre-transposes each emissions shard to [H, T, Bs] and casts to
    BF16 (halves HBM traffic; logit error ~1e-3 absolute is far inside
    the error budget of the ~5e5-magnitude output), so the device matmul
    (contract over H on partitions) needs no on-device transposes.
  - Device: logits^T[k, (t,b)] = W^T @ emisT  (PE, PSUM accumulate over 2
    h-chunks, bf16 single-pass); X = exp(logits + bias) (ACT, bias fused,
    bf16 out).  Emissions chunks stream over TWO DMA queues (sync +
    gpsimd), end chunks first, so both chains start while the middle
    chunks are still in flight.
  - Forward algorithm in the linear domain: a_t[j,b] stays transposed
    [K, Bs] so each step is ONE matmul with lhsT = exp(transitions) plus
    ONE DVE multiply by X_t.  Forward (from t=0) and backward (from
    t=511) chains interleave on PE/DVE, halving the serial latency;
    Z = alpha_255^T E (x*beta_256) finishes on host.
  - Renormalization every 8 rounds per chain keeps bf16 state in range:
    GPSIMD partition_all_reduce for the column sums (off the chain's
    PE/DVE critical path), exact power-of-two reciprocal via an
    exponent-field bitcast trick on DVE, and the scale is folded into a
    pre-multiplied X slice applied 5 rounds later; the norms are
    recorded (ACT) and folded back in on the host.
  - Gold-path logit sum is computed on the HOST (one einsum over the
    emissions shard and the gathered W columns) — cheaper than building
    a one-hot on host, and it keeps the device DVE free for the chain.
  - Host finishes: logZ_b = sum(ln s) + ln(alpha_255^T E (x*beta_256));
    numerator = host gold-logit sum + tags-only terms (start/trans/end/
    bias); final scalar = sum_b(score_b - logZ_b).
"""

import numpy as np

B, T, H, K = 256, 512, 256, 32
NCORES = 8
BS = B // NCORES          # 32 batch rows per core
NT = T * BS               # 16384 tokens per core
CHUNK = 2048              # tokens per DMA chunk
SUB = 512                 # tokens per matmul / X tile
NCHUNK = NT // CHUNK      # 8
NSUB = CHUNK // SUB       # 4
NXT = NT // SUB           # 32 X tiles
TS_PER_XT = SUB // BS     # 16 t-steps per X tile
RENORM = 8                # renormalize each chain's state every 8 rounds
NRENORM = 32              # slab slots per chain (bwd uses 31)
NROUND = 255              # bidirectional: fwd t=1..255, bwd t=510..256

_BUILT = {}
LAST_RESULTS = None


def _build_nc(parts="all"):
    import concourse.bacc as bacc
    import concourse.tile as tile
    from concourse import mybir
    from concourse import bass_isa
    from contextlib import ExitStack

    do_bulk = parts in ("all", "bulk")
    do_chain = parts in ("all", "chain", "chain_norenorm")
    do_renorm = parts in ("all", "chain")

    f32 = mybir.dt.float32
    bf16 = mybir.dt.bfloat16
    u32 = mybir.dt.uint32
    Exp = mybir.ActivationFunctionType.Exp
    Copy = mybir.ActivationFunctionType.Copy
    mult = mybir.AluOpType.mult

    nc = bacc.Bacc("TRN2", target_bir_lowering=False, debug=False,
                   num_devices=NCORES)

    emisT = nc.declare_dram_parameter("emisT", [2, 128, NT], bf16,
                                      isOutput=False)
    # packed consts: wpk = W^T halves side by side; etab = E | E^T;
    # cvec = bias | exp(start) | exp(end)
    wpk = nc.declare_dram_parameter("wpk", [128, 2 * K], bf16, isOutput=False)
    etab = nc.declare_dram_parameter("etab", [K, 2 * K], bf16, isOutput=False)
    cvec = nc.declare_dram_parameter("cvec", [K, 3], f32, isOutput=False)
    amv_d = nc.declare_dram_parameter("amv", [K, 2 * BS], bf16, isOutput=True)
    slab_d = nc.declare_dram_parameter("slab", [1, 2 * NRENORM * BS], f32,
                                       isOutput=True)

    with ExitStack() as ctx:
        tc = ctx.enter_context(tile.TileContext(nc))
        consts = ctx.enter_context(tc.tile_pool(name="consts", bufs=1))
        emis_pool = ctx.enter_context(tc.tile_pool(name="emis", bufs=NCHUNK))
        xpool = ctx.enter_context(tc.tile_pool(name="xp", bufs=NXT))
        apool = ctx.enter_context(tc.tile_pool(name="ap", bufs=16))
        bcpool = ctx.enter_context(tc.tile_pool(name="bc", bufs=3))
        psum_l = ctx.enter_context(tc.tile_pool(name="pl", bufs=4,
                                                space="PSUM"))
        psum_c = ctx.enter_context(tc.tile_pool(name="pc", bufs=1,
                                                space="PSUM"))
        psum_r = ctx.enter_context(tc.tile_pool(name="pr", bufs=1,
                                                space="PSUM"))

        # constants (packed; tiny DMAs ride the gpsimd queue so they
        # don't delay the emissions chunks on the sync queue)
        w_sb = consts.tile([128, 2 * K], bf16)
        etab_sb = consts.tile([K, 2 * K], bf16)
        cvec_sb = consts.tile([K, 3], f32)
        slab_sb = consts.tile([1, 2 * NRENORM * BS], f32)
        amv_sb = consts.tile([K, 2 * BS], bf16)
        ones_col = consts.tile([K, 1], bf16)
        ones_row = consts.tile([1, K], bf16)
        # dedicated early tiles for the two chain-init subs (t=0 is in
        # chunk0 sub0, t=511 in chunk7 sub3), loaded on the otherwise
        # idle gpsimd DMA ring so both chains can start ~8us earlier;
        # the sync-ring whole-chunk flow is left undisturbed (chunk0/7
        # redundantly reload these regions, which is harmless).
        est0 = [consts.tile([128, SUB], bf16, name=f"est0_{i}")
                for i in range(2)]
        est7 = [consts.tile([128, SUB], bf16, name=f"est7_{i}")
                for i in range(2)]
        nc.gpsimd.dma_start(out=w_sb, in_=wpk[:, :])
        nc.gpsimd.dma_start(out=cvec_sb, in_=cvec[:, :])
        c7s3 = 7 * CHUNK + 3 * SUB
        nc.gpsimd.dma_start(out=est7[0], in_=emisT[0, :, c7s3:c7s3 + SUB])
        nc.gpsimd.dma_start(out=est7[1], in_=emisT[1, :, c7s3:c7s3 + SUB])
        nc.gpsimd.dma_start(out=etab_sb, in_=etab[:, :])
        nc.gpsimd.dma_start(out=est0[0], in_=emisT[0, :, 0:SUB])
        nc.gpsimd.dma_start(out=est0[1], in_=emisT[1, :, 0:SUB])
        w0 = w_sb[:, 0:K]
        w1 = w_sb[:, K:2 * K]
        ehat_sb = etab_sb[:, 0:K]
        ebwd_sb = etab_sb[:, K:2 * K]
        b_sb = cvec_sb[:, 0:1]
        estart_sb = cvec_sb[:, 1:2]
        eend_sb = cvec_sb[:, 2:3]
        shf_sb = slab_sb[:, 0:NRENORM * BS]
        shb_sb = slab_sb[:, NRENORM * BS:2 * NRENORM * BS]

        nc.vector.memset(slab_sb, 1.0)
        nc.vector.memset(ones_col, 1.0)
        nc.vector.memset(ones_row, 1.0)

        # ---- bulk: stream emissions, logits, X = exp(logits + b) ----
        # End chunks first; chunks 0,1,2,3 on the sync DMA queue and
        # 7,6,5,4 on the gpsimd queue so the two ends load in parallel.
        xtiles = [None] * NXT
        chunk_order = [0, 7, 1, 6, 2, 5, 3, 4]
        etiles = {}
        if do_bulk:
            for c in chunk_order:
                cs, ce = c * CHUNK, (c + 1) * CHUNK
                e0 = emis_pool.tile([128, CHUNK], bf16, tag="e0")
                e1 = emis_pool.tile([128, CHUNK], bf16, tag="e1")
                nc.sync.dma_start(out=e0, in_=emisT[0, :, cs:ce])
                nc.sync.dma_start(out=e1, in_=emisT[1, :, cs:ce])
                etiles[c] = (e0, e1)
        for c in chunk_order:
            subs = range(NSUB - 1, -1, -1) if c == 7 else range(NSUB)
            for s in subs:
                xt = xpool.tile([K, SUB], bf16, tag="xt")
                xtiles[c * NSUB + s] = xt
                if not do_bulk:
                    nc.vector.memset(xt, 1.0)
                    continue
                if c == 0 and s == 0:
                    s0, s1 = est0[0], est0[1]
                elif c == 7 and s == 3:
                    s0, s1 = est7[0], est7[1]
                else:
                    e0, e1 = etiles[c]
                    s0 = e0[:, s * SUB:(s + 1) * SUB]
                    s1 = e1[:, s * SUB:(s + 1) * SUB]
                pl = psum_l.tile([K, SUB], f32, tag="pl")
                nc.tensor.matmul(pl, w0, s0, start=True, stop=False)
                nc.tensor.matmul(pl, w1, s1, start=False, stop=True)
                nc.scalar.activation(out=xt, in_=pl, func=Exp, bias=b_sb)

        # ---- bidirectional chain (bf16 states, single-pass PE matmuls):
        # forward alpha from t=0 and backward beta from t=511 run as two
        # independent 255-round recurrences that interleave on PE/DVE,
        # halving the serial latency; Z = alpha_255^T E beta-part on host ----
        def xslice(t):
            return xtiles[t // TS_PER_XT][:, (t % TS_PER_XT) * BS:
                                          (t % TS_PER_XT + 1) * BS]

        a_prev = apool.tile([K, BS], bf16, tag="af")
        nc.vector.tensor_scalar(out=a_prev, in0=xslice(0),
                                scalar1=estart_sb, scalar2=None, op0=mult)
        v_prev = apool.tile([K, BS], bf16, tag="av")
        nc.vector.tensor_scalar(out=v_prev, in0=xslice(T - 1),
                                scalar1=eend_sb, scalar2=None, op0=mult)

        if do_chain:
            # Renorm schedule, staggered so the two chains' extra helper
            # work lands on different rounds, and spread over rounds
            # r+2 / r+3 via deferred emission (per-engine queues are
            # in-order; emitting helper ops too early would head-of-line
            # block the chain ops behind them).
            # chain f: measure r%8==2, apply (pre-scaled X) r%8==7 (lag 5)
            # chain v: measure r%8==6, apply r%8==3 from r=11 (lag 5)
            states = {
                "f": dict(a=a_prev, lhs=ehat_sb, slab=shf_sb, q=[], nm=0,
                          pm=2, pa=7, amin=7, nmax=NRENORM),
                "v": dict(a=v_prev, lhs=ebwd_sb, slab=shb_sb, q=[], nm=0,
                          pm=6, pa=3, amin=11, nmax=NRENORM - 1),
            }
            deferred = {}
            for r in range(1, NROUND + 1):
                for h in ("f", "v"):
                    st = states[h]
                    t = r if h == "f" else T - 1 - r
                    if (do_renorm and r % RENORM == st["pa"]
                            and r >= st["amin"] and st["q"]):
                        xsl = st["q"].pop(0)  # X slice pre-scaled by 1/s
                    else:
                        xsl = xslice(t)
                    pc = psum_c.tile([K, BS], f32, tag="pc" + h)
                    nc.tensor.matmul(pc, st["lhs"], st["a"],
                                     start=True, stop=True)
                    a_new = apool.tile([K, BS], bf16, tag="a" + h)
                    nc.vector.tensor_mul(a_new, pc, xsl)
                    st["a"] = a_new
                    if (do_renorm and r % RENORM == st["pm"]
                            and st["nm"] < st["nmax"]):
                        slot = st["nm"]
                        st["nm"] += 1
                        ta = r + 5 if h == "f" else T - 1 - (r + 5)
                        # column sums of the fresh state via ones matmul.
                        # cs/bc psum tags are shared by both chains:
                        # their renorm events are 4 rounds apart and
                        # each tile's lifetime is <= 3 rounds.
                        cs_ps = psum_r.tile([1, BS], f32, tag="cs")
                        nc.tensor.matmul(cs_ps, ones_col, st["a"],
                                         start=True, stop=True)
                        rbc = bcpool.tile([1, BS], f32, tag="rbc" + h)
                        rbb = bcpool.tile([1, BS], bf16, tag="rbb" + h)
                        bc_ps = psum_r.tile([K, BS], f32, tag="bc")
                        xm = bcpool.tile([K, BS], bf16, tag="xm" + h)
                        st["q"].append(xm)

                        def mk(st=st, slot=slot, ta=ta, cs_ps=cs_ps,
                               rbc=rbc, rbb=rbb, bc_ps=bc_ps, xm=xm):
                            def ts_job():
                                # power-of-two reciprocal: flip the f32
                                # exponent field -> r = 2^(255-e); exact
                                # to record and to multiply.
                                nc.vector.tensor_scalar(
                                    out=rbc[:, :].bitcast(u32),
                                    in0=cs_ps[:, :].bitcast(u32),
                                    scalar1=0x7F800000, scalar2=0x7F800000,
                                    op0=mybir.AluOpType.bitwise_and,
                                    op1=mybir.AluOpType.bitwise_xor)

                            def cv_job():
                                # bf16 copy of the scale (exact: a power
                                # of two) + record it in the slab; both
                                # on the idle scalar engine
                                nc.scalar.activation(out=rbb, in_=rbc,
                                                     func=Copy)
                                nc.scalar.activation(
                                    out=st["slab"][0:1,
                                                   slot * BS:(slot + 1) * BS],
                                    in_=rbc[0:1, :], func=Copy)

                            def bc_job():
                                # broadcast [1,BS] -> [K,BS]: bf16 ones
                                # matmul is a single PE pass (an f32 one
                                # would run LOW/HIGH dual-pass)
                                nc.tensor.matmul(bc_ps, ones_row, rbb,
                                                 start=True, stop=True)

                            def xm_job():
                                nc.vector.tensor_mul(xm, xslice(ta), bc_ps)
                            return ts_job, cv_job, bc_job, xm_job

                        ts_job, cv_job, bc_job, xm_job = mk()
                        deferred.setdefault(r + 1, []).append(ts_job)
                        deferred.setdefault(r + 2, []).append(cv_job)
                        deferred.setdefault(r + 3, []).append(bc_job)
                        deferred.setdefault(r + 4, []).append(xm_job)
                # helper jobs run AFTER the round's chain ops so they
                # never sit at an engine queue head waiting on inputs
                # (head-of-line blocking the chain behind them).
                for job in deferred.pop(r, []):
                    job()
            for jobs in sorted(deferred.items()):
                for job in jobs[1]:
                    job()
            a_prev = states["f"]["a"]
            v_prev = states["v"]["a"]

        # pack outputs: amid|vmid in one bf16 tile (one DMA per queue)
        nc.scalar.activation(out=amv_sb[:, 0:BS], in_=a_prev, func=Copy)
        nc.scalar.activation(out=amv_sb[:, BS:2 * BS], in_=v_prev, func=Copy)
        nc.gpsimd.dma_start(out=amv_d[:, :], in_=amv_sb)
        nc.sync.dma_start(out=slab_d[:, :], in_=slab_sb)

    nc.compile()
    return nc


def _numpy_fallback(emissions, W, b, start_transitions, transitions,
                    end_transitions, tags, mask):
    # Exact replication of the reference semantics (used only if mask is not
    # all-ones, which the spec's input fill guarantees never happens).
    e = emissions.astype(np.float64)
    logits = e @ W.astype(np.float64) + b.astype(np.float64)
    mf = mask.astype(np.float64)
    st = start_transitions.astype(np.float64)
    tr = transitions.astype(np.float64)
    en = end_transitions.astype(np.float64)
    Bn = logits.shape[0]
    bar = np.arange(Bn)
    first = tags[:, 0]
    score = st[first] + logits[bar, 0, first]
    prev = first.copy()
    for t in range(1, T):
        tg = tags[:, t]
        stepv = tr[prev, tg] + logits[bar, t, tg]
        score = score + stepv * mf[:, t]
        prev = np.where(mf[:, t] > 0, tg, prev)
    score = score + en[prev]
    alpha = st[None, :] + logits[:, 0]
    for t in range(1, T):
        nxt = alpha[:, :, None] + tr[None, :, :]
        m = nxt.max(axis=1, keepdims=True)
        nxt = np.log(np.exp(nxt - m).sum(axis=1)) + m[:, 0, :] + logits[:, t]
        alpha = np.where(mf[:, t:t + 1] > 0, nxt, alpha)
    fin = alpha + en[None, :]
    m = fin.max(axis=1, keepdims=True)
    logz = np.log(np.exp(fin - m).sum(axis=1)) + m[:, 0]
    return np.asarray((score - logz).sum(), dtype=np.float32)


def kernel(emissions, W, b, start_transitions, transitions, end_transitions,
           tags, mask):
    global LAST_RESULTS
    emissions = np.ascontiguousarray(np.asarray(emissions, dtype=np.float32))
    W = np.asarray(W, dtype=np.float32)
    b = np.asarray(b, dtype=np.float32)
    start_transitions = np.asarray(start_transitions, dtype=np.float32)
    transitions = np.asarray(transitions, dtype=np.float32)
    end_transitions = np.asarray(end_transitions, dtype=np.float32)
    tags = np.asarray(tags).astype(np.int64)
    mask = np.asarray(mask).astype(bool)

    if not mask.all():
        return _numpy_fallback(emissions, W, b, start_transitions, transitions,
                               end_transitions, tags, mask)

    from concourse.bass_utils import run_bass_kernel_spmd

    if "nc" not in _BUILT:
        _BUILT["nc"] = _build_nc()
    nc = _BUILT["nc"]

    import ml_dtypes
    bf = ml_dtypes.bfloat16
    wpk_h = np.ascontiguousarray(
        W.reshape(2, 128, K).transpose(1, 0, 2).reshape(128, 2 * K).astype(bf))
    E32 = np.exp(transitions).astype(np.float32)
    etab_h = np.ascontiguousarray(
        np.concatenate([E32, E32.T], axis=1).astype(bf))
    cvec_h = np.ascontiguousarray(np.stack(
        [b, np.exp(start_transitions), np.exp(end_transitions)],
        axis=1).astype(np.float32))

    in_maps = []
    emisT_f32 = []
    for c in range(NCORES):
        sh = emissions[c * BS:(c + 1) * BS]              # [BS, T, H]
        shT = np.ascontiguousarray(sh.transpose(2, 1, 0))  # [H, T, BS]
        emisT_f32.append(shT)
        emisT_h = shT.astype(bf).reshape(2, 128, NT)
        in_maps.append(dict(emisT=emisT_h, wpk=wpk_h, etab=etab_h,
                            cvec=cvec_h))

    res = run_bass_kernel_spmd(nc, in_maps, list(range(NCORES)))
    LAST_RESULTS = res

    E64 = np.exp(transitions.astype(np.float64))
    total = 0.0
    for c in range(NCORES):
        out = res.results[c]
        amv = np.asarray(out["amv"]).astype(np.float64)  # [K, 2*BS]
        amid = amv[:, 0:BS]                              # alpha_255
        vmid = amv[:, BS:2 * BS]                         # x_256*beta_256
        slab = out["slab"].astype(np.float64).reshape(2, NRENORM, BS)
        # Z_b = alpha_255^T E (x_256*beta_256), scaled by recorded norms
        zmid = np.einsum("kb,kj,jb->b", amid, E64, vmid)
        logz = (-np.log(slab[0]).sum(axis=0) - np.log(slab[1]).sum(axis=0)
                + np.log(zmid))
        tg = tags[c * BS:(c + 1) * BS]
        # gold-path logit sum on host: sum_{b,t} emis[b,t,:] . W[:,tag]
        tgflat = tg.T.reshape(-1)                        # t-major, matches NT
        ef = emisT_f32[c].reshape(H, NT)
        gold = np.einsum("hc,hc->", ef, W[:, tgflat], dtype=np.float64)
        hterm = (start_transitions.astype(np.float64)[tg[:, 0]].sum()
                 + transitions.astype(np.float64)[tg[:, :-1], tg[:, 1:]].sum()
                 + end_transitions.astype(np.float64)[tg[:, -1]].sum()
                 + b.astype(np.float64)[tg].sum())
        total += gold + hterm - logz.sum()

    return np.asarray(total, dtype=np.float32)
